# revision 16
# baseline (speedup 1.0000x reference)
"""PerformerAttention (standard softmax attention + interleaved RoPE) on 8 trn2 cores.

Sharding: data-parallel over batch (16 batches -> 2 per core), weights replicated.

Math per core (b = 2 local batches):
  qkv^T = wqkv @ x^T          (PE, fp32 for q/k path; V computed separately in bf16)
  RoPE on q,k (tokens 1..576) -- the even/odd interleave is pre-permuted into
    the wqkv rows host-side so on-chip rope uses contiguous 32-row blocks.
  S = (q/8)^T k  -> softmax over free dim  (fp32 scores: argmax-critical)
  P transposed via PE; out^T = V^T-style accumulation  (bf16)
  final = out^T.T @ wproj^T + bias  (bf16 matmul, fp32 accum)

Host-side prep (numpy): shard x, transpose x/w, permute wqkv q/k rows,
build cos/sin tiles, broadcast bias, cast bf16 operands.
"""

import numpy as np
import ml_dtypes

import concourse.bass as bass
import concourse.mybir as mybir
import concourse.tile as tile
from concourse import bacc
from concourse.bass_utils import run_bass_kernel_spmd
from concourse.masks import make_identity

F32 = mybir.dt.float32
F32R = mybir.dt.float32r
BF16 = mybir.dt.bfloat16
AX = mybir.AxisListType.X
COPY = mybir.ActivationFunctionType.Copy
EXP = mybir.ActivationFunctionType.Exp

B, N, C, H, D = 16, 577, 768, 12, 64
NCORES = 8
BPC = B // NCORES  # batches per core
NM1 = N - 1  # 576, rope'd tokens
NP = 578  # padded N (fp32r needs even free sizes)
NT = [(0, 128), (128, 128), (256, 128), (384, 128), (512, 65)]  # n tiles
MCH = [(0, 512), (512, 65)]  # free-dim chunks (psum bank = 512 fp32)
SCH = [(0, 320), (320, 258)]  # fp32r chunks (>=256 for 1 cycle/row, even)
FCH = [(0, 384), (384, 384)]  # proj/V output chunks

_CACHED_NC = None
LAST_RESULTS = None  # test harness reads exec_time_ns off this


def _build_nc():
    nc = bacc.Bacc("TRN2", target_bir_lowering=False)

    xT_d = nc.dram_tensor("xT", [BPC, C, NP], F32R, kind="ExternalInput")
    xTv_d = nc.dram_tensor("xTv", [BPC, C, N], BF16, kind="ExternalInput")
    wqk_d = nc.dram_tensor("wqkT", [C, 2 * C], F32R, kind="ExternalInput")
    wv_d = nc.dram_tensor("wvT", [C, C], BF16, kind="ExternalInput")
    wp_d = nc.dram_tensor("wpT", [C, C], BF16, kind="ExternalInput")
    c_d = nc.dram_tensor("c128", [128, NM1], F32, kind="ExternalInput")
    s_d = nc.dram_tensor("s128", [128, NM1], F32, kind="ExternalInput")
    bias_d = nc.dram_tensor("biasb", [128, C], F32, kind="ExternalInput")
    out_d = nc.dram_tensor("out", [BPC, N, C], F32, kind="ExternalOutput")

    with tile.TileContext(nc) as tc:
        with (
            tc.tile_pool(name="const", bufs=1) as constp,
            tc.tile_pool(name="persist", bufs=1) as pers,
            tc.tile_pool(name="work", bufs=2) as work,
            tc.tile_pool(name="ppool", bufs=6) as ppool,
            tc.tile_pool(name="ptpool", bufs=5) as ptpool,
            tc.tile_pool(name="obp", bufs=3) as obp,
            tc.tile_pool(name="stp", bufs=10) as stp,
            tc.tile_pool(name="psum", bufs=4, space="PSUM") as psp,
        ):
            # ---- constants / weights (once per core) ----
            ident = constp.tile([128, 128], BF16, name="ident", tag="ident")
            make_identity(nc, ident)

            wqk = []
            wv = []
            wp = []
            for ct in range(6):
                t = constp.tile([128, 2 * C], F32R, name=f"wqk{ct}", tag=f"wqk{ct}")
                nc.sync.dma_start(t, wqk_d[ct * 128:(ct + 1) * 128, :])
                wqk.append(t)
                t = constp.tile([128, C], BF16, name=f"wv{ct}", tag=f"wv{ct}")
                nc.sync.dma_start(t, wv_d[ct * 128:(ct + 1) * 128, :])
                wv.append(t)
                t = constp.tile([128, C], BF16, name=f"wp{ct}", tag=f"wp{ct}")
                nc.sync.dma_start(t, wp_d[ct * 128:(ct + 1) * 128, :])
                wp.append(t)
            c128 = constp.tile([128, NM1], F32, name="c128", tag="c128")
            nc.sync.dma_start(c128, c_d[:, :])
            s128 = constp.tile([128, NM1], F32, name="s128", tag="s128")
            nc.sync.dma_start(s128, s_d[:, :])
            biasb = constp.tile([128, C], F32, name="biasb", tag="biasb")
            nc.sync.dma_start(biasb, bias_d[:, :])

            # persistent per-batch tiles
            xT = [pers.tile([128, NP], F32R, name=f"xT{ct}", tag=f"xT{ct}")
                  for ct in range(6)]
            xTv = [pers.tile([128, N], BF16, name=f"xTv{ct}", tag=f"xTv{ct}")
                   for ct in range(6)]
            qkT = [pers.tile([128, NP], F32R, name=f"qkT{ft}", tag=f"qkT{ft}")
                   for ft in range(12)]
            V = [pers.tile([128, C], BF16, name=f"V{nt}", tag=f"V{nt}")
                 for nt in range(5)]
            attnT = [pers.tile([128, N], BF16, name=f"attnT{ct}", tag=f"attnT{ct}")
                     for ct in range(6)]

            for b in range(BPC):
                # ---- load x^T (fp32 + bf16 copies) ----
                for ct in range(6):
                    nc.sync.dma_start(xT[ct], xT_d[b, ct * 128:(ct + 1) * 128, :])
                    nc.sync.dma_start(xTv[ct], xTv_d[b, ct * 128:(ct + 1) * 128, :])

                # ---- qk^T = wqk^T.T @ x^T   [f,n] fp32r ----
                for ft in range(12):
                    for (cs, cw) in SCH:
                        ps = psp.tile([128, 512], F32, name="ps_qk", tag="mm")
                        for ct in range(6):
                            nc.tensor.matmul(
                                ps[:, 0:cw],
                                lhsT=wqk[ct][:, ft * 128:(ft + 1) * 128],
                                rhs=xT[ct][:, cs:cs + cw],
                                start=(ct == 0), stop=(ct == 5),
                            )
                        nc.scalar.activation(
                            qkT[ft][:, cs:cs + cw], ps[:, 0:cw], COPY,
                            bias=0.0, scale=1.0,
                        )

                # ---- RoPE on q,k tiles (cols 1..576), blocks [te;to;te;to] ----
                for ft in range(12):
                    t = qkT[ft]
                    tsw = work.tile([128, NM1], F32, name="tsw", tag="tsw")
                    rot1 = work.tile([128, NM1], F32, name="rot1", tag="rot1")
                    # tsw = [to0; te0; to1; te1]
                    nc.gpsimd.tensor_copy(tsw[0:32, :], t[32:64, 1:N])
                    nc.gpsimd.tensor_copy(tsw[32:64, :], t[0:32, 1:N])
                    nc.gpsimd.tensor_copy(tsw[64:96, :], t[96:128, 1:N])
                    nc.gpsimd.tensor_copy(tsw[96:128, :], t[64:96, 1:N])
                    nc.vector.tensor_mul(rot1, t[:, 1:N], c128)
                    nc.vector.tensor_mul(tsw, tsw, s128)  # s128 = [-s;s;-s;s]
                    nc.vector.tensor_add(t[:, 1:N], rot1, tsw)

                # ---- V = x @ wv^T   [n,f] bf16 ----
                for (ns, nsz) in NT:
                    nt = ns // 128
                    for (fs, fw) in FCH:
                        ps = psp.tile([128, 512], F32, name="ps_v", tag="mm")
                        for ct in range(6):
                            nc.tensor.matmul(
                                ps[0:nsz, 0:fw],
                                lhsT=xTv[ct][:, ns:ns + nsz],
                                rhs=wv[ct][:, fs:fs + fw],
                                start=(ct == 0), stop=(ct == 5),
                            )
                        nc.vector.tensor_copy(V[nt][0:nsz, fs:fs + fw],
                                              ps[0:nsz, 0:fw])

                # ---- attention per head ----
                for h in range(12):
                    qt = qkT[h // 2][(h % 2) * 64:(h % 2) * 64 + 64, :]
                    kt = qkT[6 + h // 2][(h % 2) * 64:(h % 2) * 64 + 64, :]

                    P = []
                    for (ns, nsz) in NT:
                        st = stp.tile([128, 8], F32, name="st", tag="st")
                        psA = psp.tile([128, 512], F32, name="ps_sA", tag="mm")
                        psB = psp.tile([128, 512], F32, name="ps_sB", tag="mm")
                        nc.tensor.matmul(psA[0:nsz, 0:320],
                                         lhsT=qt[:, ns:ns + nsz],
                                         rhs=kt[:, 0:320],
                                         start=True, stop=True)
                        nc.tensor.matmul(psB[0:nsz, 0:258],
                                         lhsT=qt[:, ns:ns + nsz],
                                         rhs=kt[:, 320:578],
                                         start=True, stop=True)
                        nc.vector.reduce_max(st[0:nsz, 0:1], psA[0:nsz, 0:320],
                                             axis=AX)
                        nc.vector.reduce_max(st[0:nsz, 1:2], psB[0:nsz, 0:257],
                                             axis=AX)
                        nc.vector.tensor_max(st[0:nsz, 2:3], st[0:nsz, 0:1],
                                             st[0:nsz, 1:2])
                        nc.vector.tensor_scalar_mul(st[0:nsz, 3:4],
                                                    st[0:nsz, 2:3], -1.0)
                        Pt = ppool.tile([128, N], BF16, name="P", tag="P")
                        nc.scalar.activation(Pt[0:nsz, 0:320], psA[0:nsz, 0:320],
                                             EXP, bias=st[0:nsz, 3:4],
                                             accum_out=st[0:nsz, 4:5])
                        nc.scalar.activation(Pt[0:nsz, 320:577], psB[0:nsz, 0:257],
                                             EXP, bias=st[0:nsz, 3:4],
                                             accum_out=st[0:nsz, 5:6])
                        nc.vector.tensor_add(st[0:nsz, 6:7], st[0:nsz, 4:5],
                                             st[0:nsz, 5:6])
                        nc.vector.reciprocal(st[0:nsz, 7:8], st[0:nsz, 6:7])
                        nc.gpsimd.tensor_scalar_mul(Pt[0:nsz, :], Pt[0:nsz, :],
                                                    st[0:nsz, 7:8])
                        P.append(Pt)

                    # transpose P -> PT[mt] [m, n] bf16
                    PT = []
                    for (ms, msz) in NT:
                        psT = psp.tile([128, 512], BF16, name="psT", tag="tr",
                                       bufs=2)
                        psT4 = psp.tile([128, 128], BF16, name="psT4", tag="tr2",
                                        bufs=2)
                        for (ns, nsz) in NT:
                            nt = ns // 128
                            dst = (psT[0:msz, ns:ns + nsz] if nt < 4
                                   else psT4[0:msz, 0:nsz])
                            nc.tensor.transpose(dst, P[nt][0:nsz, ms:ms + msz],
                                                ident[0:nsz, 0:nsz])
                        PTt = ptpool.tile([128, N], BF16, name="PT", tag="PT")
                        nc.vector.tensor_copy(PTt[0:msz, 0:512], psT[0:msz, :])
                        nc.vector.tensor_copy(PTt[0:msz, 512:577],
                                              psT4[0:msz, 0:65])
                        PT.append(PTt)

                    # out^T[h] = sum_m V[m,:]^T-style: lhsT=V tile, rhs=PT
                    for (cs, cw) in MCH:
                        pso = psp.tile([128, 512], F32, name="ps_o", tag="mm")
                        for (ms, msz) in NT:
                            mt = ms // 128
                            nc.tensor.matmul(
                                pso[0:64, 0:cw],
                                lhsT=V[mt][0:msz, h * 64:h * 64 + 64],
                                rhs=PT[mt][0:msz, cs:cs + cw],
                                start=(mt == 0), stop=(mt == 4),
                            )
                        nc.vector.tensor_copy(
                            attnT[h // 2][(h % 2) * 64:(h % 2) * 64 + 64,
                                          cs:cs + cw],
                            pso[0:64, 0:cw])

                # ---- proj + bias ----
                for (ns, nsz) in NT:
                    ob = obp.tile([128, C], F32, name="ob", tag="ob")
                    for (fs, fw) in FCH:
                        ps = psp.tile([128, 512], F32, name="ps_p", tag="mm")
                        for ct in range(6):
                            nc.tensor.matmul(
                                ps[0:nsz, 0:fw],
                                lhsT=attnT[ct][:, ns:ns + nsz],
                                rhs=wp[ct][:, fs:fs + fw],
                                start=(ct == 0), stop=(ct == 5),
                            )
                        nc.vector.tensor_add(ob[0:nsz, fs:fs + fw],
                                             ps[0:nsz, 0:fw],
                                             biasb[0:nsz, fs:fs + fw])
                    nc.sync.dma_start(out_d[b, ns:ns + nsz, :], ob[0:nsz, :])

    nc.compile()
    return nc


def _rope_perm():
    idx = []
    for h in range(H):
        base = h * D
        idx.extend(base + 2 * i for i in range(D // 2))      # evens
        idx.extend(base + 2 * i + 1 for i in range(D // 2))  # odds
    return np.array(idx)


def _prep_inputs(x, wqkv, wproj, bproj, freqs_cos, freqs_sin):
    perm = _rope_perm()
    wq = wqkv[0:C][perm] * 0.125
    wk = wqkv[C:2 * C][perm]
    wqkT = np.ascontiguousarray(np.concatenate([wq, wk], axis=0).T,
                                dtype=np.float32)
    wvT = np.ascontiguousarray(wqkv[2 * C:].T).astype(ml_dtypes.bfloat16)
    wpT = np.ascontiguousarray(wproj.T).astype(ml_dtypes.bfloat16)
    cosT = np.ascontiguousarray(freqs_cos.T, dtype=np.float32)  # [32, 576]
    sinT = np.ascontiguousarray(freqs_sin.T, dtype=np.float32)
    c128 = np.concatenate([cosT] * 4, axis=0)
    s128 = np.concatenate([-sinT, sinT, -sinT, sinT], axis=0)
    biasb = np.broadcast_to(bproj.astype(np.float32), (128, C)).copy()

    in_maps = []
    for core in range(NCORES):
        xs = x[core * BPC:(core + 1) * BPC]
        xT = np.ascontiguousarray(xs.transpose(0, 2, 1), dtype=np.float32)
        xTp = np.zeros((BPC, C, NP), dtype=np.float32)
        xTp[:, :, 0:N] = xT
        in_maps.append({
            "xT": xTp,
            "xTv": xT.astype(ml_dtypes.bfloat16),
            "wqkT": wqkT,
            "wvT": wvT,
            "wpT": wpT,
            "c128": c128,
            "s128": s128,
            "biasb": biasb,
        })
    return in_maps


def kernel(x, wqkv, wproj, bproj, freqs_cos, freqs_sin, trace=False):
    global _CACHED_NC, LAST_RESULTS
    if _CACHED_NC is None:
        _CACHED_NC = _build_nc()
    in_maps = _prep_inputs(x, wqkv, wproj, bproj, freqs_cos, freqs_sin)
    res = run_bass_kernel_spmd(_CACHED_NC, in_maps,
                               core_ids=list(range(NCORES)), trace=trace)
    LAST_RESULTS = res
    out = np.concatenate([r["out"] for r in res.results], axis=0)
    return out.astype(np.float32)


# revision 25
# speedup vs baseline: 1.8669x; 1.8669x over previous
"""PerformerAttention (standard softmax attention + interleaved RoPE) on 8 trn2 cores.

Sharding: data-parallel over batch (16 batches -> 2 per core), weights replicated.

Math per core (b = 2 local batches):
  qkv^T = wqkv @ x^T          (PE, fp32 for q/k path; V computed separately in bf16)
  RoPE on q,k (tokens 1..576) -- the even/odd interleave is pre-permuted into
    the wqkv rows host-side so on-chip rope uses contiguous 32-row blocks.
  S = (q/8)^T k  -> softmax over free dim  (fp32 scores: argmax-critical)
  P transposed via PE; out^T = V^T-style accumulation  (bf16)
  final = out^T.T @ wproj^T + bias  (bf16 matmul, fp32 accum)

Host-side prep (numpy): shard x, transpose x/w, permute wqkv q/k rows,
build cos/sin tiles, broadcast bias, cast bf16 operands.
"""

import numpy as np
import ml_dtypes

import concourse.bass as bass
import concourse.mybir as mybir
import concourse.tile as tile
from concourse import bacc
from concourse.bass_utils import run_bass_kernel_spmd
from concourse.masks import make_identity

F32 = mybir.dt.float32
F32R = mybir.dt.float32r
BF16 = mybir.dt.bfloat16
AX = mybir.AxisListType.X
COPY = mybir.ActivationFunctionType.Copy
EXP = mybir.ActivationFunctionType.Exp
LN = mybir.ActivationFunctionType.Ln
from concourse.alu_op_type import AluOpType
MIN = AluOpType.min

B, N, C, H, D = 16, 577, 768, 12, 64
NCORES = 8
BPC = B // NCORES  # batches per core
NM1 = N - 1  # 576, rope'd tokens
NP = 578  # padded N (fp32r needs even free sizes)
NT = [(0, 128), (128, 128), (256, 128), (384, 128), (512, 65)]  # n tiles
MCH = [(0, 512), (512, 65)]  # free-dim chunks (psum bank = 512 fp32)
SCH = [(0, 320), (320, 258)]  # fp32r chunks (>=256 for 1 cycle/row, even)
FCH = [(0, 384), (384, 384)]  # proj/V output chunks

_CACHED_NC = None
LAST_RESULTS = None  # test harness reads exec_time_ns off this


def _build_nc():
    nc = bacc.Bacc("TRN2", target_bir_lowering=False)

    xT_d = nc.dram_tensor("xT", [BPC, C, NP], F32R, kind="ExternalInput")
    xTv_d = nc.dram_tensor("xTv", [BPC, C, N], BF16, kind="ExternalInput")
    wqk_d = nc.dram_tensor("wqkT", [C, 2 * C], F32R, kind="ExternalInput")
    wv_d = nc.dram_tensor("wvT", [C, C], BF16, kind="ExternalInput")
    wp_d = nc.dram_tensor("wpT", [C, C], BF16, kind="ExternalInput")
    c_d = nc.dram_tensor("c128", [128, NM1], F32, kind="ExternalInput")
    s_d = nc.dram_tensor("s128", [128, NM1], F32, kind="ExternalInput")
    bias_d = nc.dram_tensor("biasb", [128, C], F32, kind="ExternalInput")
    out_d = nc.dram_tensor("out", [BPC, N, C], F32, kind="ExternalOutput")

    with tile.TileContext(nc) as tc:
        with (
            tc.tile_pool(name="const", bufs=1) as constp,
            tc.tile_pool(name="persist", bufs=1) as pers,
            tc.tile_pool(name="work", bufs=2) as work,
            tc.tile_pool(name="ppool", bufs=6) as ppool,
            tc.tile_pool(name="ptpool", bufs=5) as ptpool,
            tc.tile_pool(name="obp", bufs=3) as obp,
            tc.tile_pool(name="psum", bufs=4, space="PSUM") as psp,
        ):
            # ---- constants / weights (once per core) ----
            ident = constp.tile([128, 128], BF16, name="ident", tag="ident")
            make_identity(nc, ident)
            identf = constp.tile([128, 128], F32, name="identf", tag="identf")
            make_identity(nc, identf)
            ones1 = constp.tile([1, 128], F32, name="ones1", tag="ones1")
            nc.vector.memset(ones1, 1.0)

            wqk = []
            wv = []
            wp = []
            for ct in range(6):
                t = constp.tile([128, 2 * C], F32R, name=f"wqk{ct}", tag=f"wqk{ct}")
                nc.sync.dma_start(t, wqk_d[ct * 128:(ct + 1) * 128, :])
                wqk.append(t)
                t = constp.tile([128, C], BF16, name=f"wv{ct}", tag=f"wv{ct}")
                nc.sync.dma_start(t, wv_d[ct * 128:(ct + 1) * 128, :])
                wv.append(t)
                t = constp.tile([128, C], BF16, name=f"wp{ct}", tag=f"wp{ct}")
                nc.sync.dma_start(t, wp_d[ct * 128:(ct + 1) * 128, :])
                wp.append(t)
            c128 = constp.tile([128, NM1], F32, name="c128", tag="c128")
            nc.sync.dma_start(c128, c_d[:, :])
            s128 = constp.tile([128, NM1], F32, name="s128", tag="s128")
            nc.sync.dma_start(s128, s_d[:, :])
            biasb = constp.tile([128, C], F32, name="biasb", tag="biasb")
            nc.sync.dma_start(biasb, bias_d[:, :])

            # persistent per-batch tiles
            xT = [pers.tile([128, NP], F32R, name=f"xT{ct}", tag=f"xT{ct}")
                  for ct in range(6)]
            xTv = [pers.tile([128, N], BF16, name=f"xTv{ct}", tag=f"xTv{ct}")
                   for ct in range(6)]
            qkT = [pers.tile([128, NP], F32R, name=f"qkT{ft}", tag=f"qkT{ft}")
                   for ft in range(12)]
            V = [pers.tile([128, C], BF16, name=f"V{nt}", tag=f"V{nt}")
                 for nt in range(5)]
            attnT = [pers.tile([128, N], BF16, name=f"attnT{ct}", tag=f"attnT{ct}")
                     for ct in range(6)]

            for b in range(BPC):
                # ---- load x^T (fp32 + bf16 copies) ----
                for ct in range(6):
                    nc.sync.dma_start(xT[ct], xT_d[b, ct * 128:(ct + 1) * 128, :])
                    nc.sync.dma_start(xTv[ct], xTv_d[b, ct * 128:(ct + 1) * 128, :])

                # ---- qk^T = wqk^T.T @ x^T   [f,n] fp32r ----
                for ft in range(12):
                    for (cs, cw) in SCH:
                        ps = psp.tile([128, 512], F32, name="ps_qk", tag="mm")
                        for ct in range(6):
                            nc.tensor.matmul(
                                ps[:, 0:cw],
                                lhsT=wqk[ct][:, ft * 128:(ft + 1) * 128],
                                rhs=xT[ct][:, cs:cs + cw],
                                start=(ct == 0), stop=(ct == 5),
                            )
                        nc.scalar.activation(
                            qkT[ft][:, cs:cs + cw], ps[:, 0:cw], COPY,
                            bias=0.0, scale=1.0,
                        )

                # ---- RoPE on q,k tiles (cols 1..576), blocks [te;to;te;to] ----
                for ft in range(12):
                    t = qkT[ft]
                    tsw = work.tile([128, NM1], F32, name="tsw", tag="tsw")
                    rot1 = work.tile([128, NM1], F32, name="rot1", tag="rot1")
                    # tsw = [to0; te0; to1; te1]
                    nc.vector.tensor_copy(tsw[0:32, :], t[32:64, 1:N])
                    nc.vector.tensor_copy(tsw[32:64, :], t[0:32, 1:N])
                    nc.vector.tensor_copy(tsw[64:96, :], t[96:128, 1:N])
                    nc.vector.tensor_copy(tsw[96:128, :], t[64:96, 1:N])
                    nc.vector.tensor_mul(rot1, t[:, 1:N], c128)
                    nc.vector.tensor_mul(tsw, tsw, s128)  # s128 = [-s;s;-s;s]
                    nc.vector.tensor_add(t[:, 1:N], rot1, tsw)

                # ---- stats tiles (per batch), col = nt*12 + h ----
                stNM = pers.tile([128, 60], F32, name="stNM", tag="stNM")
                stMA = pers.tile([128, 60], F32, name="stMA", tag="stMA")
                stMB = pers.tile([128, 60], F32, name="stMB", tag="stMB")
                stSA = pers.tile([128, 60], F32, name="stSA", tag="stSA")
                stSB = pers.tile([128, 60], F32, name="stSB", tag="stSB")
                stSum = pers.tile([128, 60], F32, name="stSum", tag="stSum")
                stL = pers.tile([128, 60], F32, name="stL", tag="stL")
                stB = pers.tile([128, 60], F32, name="stB", tag="stB")
                stRow = pers.tile([12, 640], F32, name="stRow", tag="stRow")
                stRow0 = pers.tile([1, 7680], F32, name="stRow0", tag="stRow0")

                # ---- stats pass: S = q^T k [n, m]; -max and sum per row ----
                for h in range(12):
                    qt = qkT[h // 2][(h % 2) * 64:(h % 2) * 64 + 64, :]
                    kt = qkT[6 + h // 2][(h % 2) * 64:(h % 2) * 64 + 64, :]
                    for (ns, nsz) in NT:
                        nt = ns // 128
                        c = nt * 12 + h
                        psA = psp.tile([128, 512], F32, name="ps_sA", tag="mm")
                        psB = psp.tile([128, 512], F32, name="ps_sB", tag="mm")
                        nc.tensor.matmul(psA[0:nsz, 0:320],
                                         lhsT=qt[:, ns:ns + nsz],
                                         rhs=kt[:, 0:320],
                                         start=True, stop=True)
                        nc.tensor.matmul(psB[0:nsz, 0:258],
                                         lhsT=qt[:, ns:ns + nsz],
                                         rhs=kt[:, 320:578],
                                         start=True, stop=True)
                        nc.vector.reduce_max(stMA[0:nsz, c:c + 1],
                                             psA[0:nsz, 0:320], axis=AX,
                                             negate=True)
                        nc.vector.reduce_max(stMB[0:nsz, c:c + 1],
                                             psB[0:nsz, 0:257], axis=AX,
                                             negate=True)
                        nc.vector.tensor_tensor(stNM[0:nsz, c:c + 1],
                                                stMA[0:nsz, c:c + 1],
                                                stMB[0:nsz, c:c + 1], MIN)
                        scr = ppool.tile([128, 320], BF16, name="scr",
                                         tag="scr")
                        nc.scalar.activation(scr[0:nsz, 0:320],
                                             psA[0:nsz, 0:320],
                                             EXP, bias=stNM[0:nsz, c:c + 1],
                                             accum_out=stSA[0:nsz, c:c + 1])
                        nc.scalar.activation(scr[0:nsz, 0:257],
                                             psB[0:nsz, 0:257],
                                             EXP, bias=stNM[0:nsz, c:c + 1],
                                             accum_out=stSB[0:nsz, c:c + 1])

                # ---- V = x @ wv^T   [n,f] bf16 (fills PE gap) ----
                for (ns, nsz) in NT:
                    nt = ns // 128
                    for (fs, fw) in FCH:
                        ps = psp.tile([128, 512], F32, name="ps_v", tag="mm")
                        for ct in range(6):
                            nc.tensor.matmul(
                                ps[0:nsz, 0:fw],
                                lhsT=xTv[ct][:, ns:ns + nsz],
                                rhs=wv[ct][:, fs:fs + fw],
                                start=(ct == 0), stop=(ct == 5),
                            )
                        nc.vector.tensor_copy(V[nt][0:nsz, fs:fs + fw],
                                              ps[0:nsz, 0:fw])

                # ---- bias row: stB = -(max + ln(sum)), to [12, 640] ----
                nc.vector.tensor_add(stSum, stSA, stSB)
                nc.scalar.activation(stL, stSum, LN, bias=0.0, scale=1.0)
                nc.vector.memset(stB[64:128, 48:60], -30000.0)
                nc.vector.tensor_sub(stB[:, 0:48], stNM[:, 0:48],
                                     stL[:, 0:48])
                nc.vector.tensor_sub(stB[0:64, 48:60], stNM[0:64, 48:60],
                                     stL[0:64, 48:60])
                nc.vector.tensor_sub(stB[64:65, 48:60], stNM[64:65, 48:60],
                                     stL[64:65, 48:60])
                psR1 = psp.tile([12, 384], F32, name="psR1", tag="trB", bufs=2)
                psR2 = psp.tile([12, 384], F32, name="psR2", tag="trB", bufs=2)
                for nt in range(5):
                    dst = (psR1[0:12, nt * 128:(nt + 1) * 128] if nt < 3
                           else psR2[0:12, (nt - 3) * 128:(nt - 2) * 128])
                    nc.tensor.transpose(dst, stB[:, nt * 12:(nt + 1) * 12],
                                        identf)
                nc.vector.tensor_copy(stRow[0:12, 0:384], psR1[0:12, 0:384])
                nc.vector.tensor_copy(stRow[0:12, 384:640], psR2[0:12, 0:256])
                nc.sync.dma_start(stRow0[0:1, :], stRow[:, :])

                # ---- S^T pass: P^T = exp(k^T q + bias_row), then PV ----
                for h in range(12):
                    qt = qkT[h // 2][(h % 2) * 64:(h % 2) * 64 + 64, :]
                    kt = qkT[6 + h // 2][(h % 2) * 64:(h % 2) * 64 + 64, :]

                    PT = []
                    for (ms, msz) in NT:
                        PTt = ptpool.tile([128, NP], BF16, name="PT", tag="PT")
                        for (cs, cw) in SCH:
                            ps = psp.tile([128, 512], F32, name="ps_st",
                                          tag="mm")
                            nc.tensor.matmul(ps[0:msz, 0:cw],
                                             lhsT=kt[:, ms:ms + msz],
                                             rhs=qt[:, cs:cs + cw],
                                             start=True, stop=False)
                            nc.tensor.matmul(
                                ps[0:msz, 0:cw],
                                lhsT=ones1[0:1, 0:msz],
                                rhs=stRow0[0:1,
                                           h * 640 + cs:h * 640 + cs + cw],
                                start=False, stop=True)
                            nc.scalar.activation(PTt[0:msz, cs:cs + cw],
                                                 ps[0:msz, 0:cw],
                                                 EXP, bias=0.0, scale=1.0)
                        PT.append(PTt)

                    # out^T[h] = sum_m V[m,:]^T-style: lhsT=V tile, rhs=PT
                    for (cs, cw) in MCH:
                        pso = psp.tile([128, 512], F32, name="ps_o", tag="mm")
                        for (ms, msz) in NT:
                            mt = ms // 128
                            nc.tensor.matmul(
                                pso[0:64, 0:cw],
                                lhsT=V[mt][0:msz, h * 64:h * 64 + 64],
                                rhs=PT[mt][0:msz, cs:cs + cw],
                                start=(mt == 0), stop=(mt == 4),
                            )
                        nc.vector.tensor_copy(
                            attnT[h // 2][(h % 2) * 64:(h % 2) * 64 + 64,
                                          cs:cs + cw],
                            pso[0:64, 0:cw])

                # ---- proj + bias ----
                for (ns, nsz) in NT:
                    ob = obp.tile([128, C], F32, name="ob", tag="ob")
                    for (fs, fw) in FCH:
                        ps = psp.tile([128, 512], F32, name="ps_p", tag="mm")
                        for ct in range(6):
                            nc.tensor.matmul(
                                ps[0:nsz, 0:fw],
                                lhsT=attnT[ct][:, ns:ns + nsz],
                                rhs=wp[ct][:, fs:fs + fw],
                                start=(ct == 0), stop=(ct == 5),
                            )
                        nc.vector.tensor_add(ob[0:nsz, fs:fs + fw],
                                             ps[0:nsz, 0:fw],
                                             biasb[0:nsz, fs:fs + fw])
                    nc.sync.dma_start(out_d[b, ns:ns + nsz, :], ob[0:nsz, :])

    nc.compile()
    return nc


def _rope_perm():
    idx = []
    for h in range(H):
        base = h * D
        idx.extend(base + 2 * i for i in range(D // 2))      # evens
        idx.extend(base + 2 * i + 1 for i in range(D // 2))  # odds
    return np.array(idx)


def _prep_inputs(x, wqkv, wproj, bproj, freqs_cos, freqs_sin):
    perm = _rope_perm()
    wq = wqkv[0:C][perm] * 0.125
    wk = wqkv[C:2 * C][perm]
    wqkT = np.ascontiguousarray(np.concatenate([wq, wk], axis=0).T,
                                dtype=np.float32)
    wvT = np.ascontiguousarray(wqkv[2 * C:].T).astype(ml_dtypes.bfloat16)
    wpT = np.ascontiguousarray(wproj.T).astype(ml_dtypes.bfloat16)
    cosT = np.ascontiguousarray(freqs_cos.T, dtype=np.float32)  # [32, 576]
    sinT = np.ascontiguousarray(freqs_sin.T, dtype=np.float32)
    c128 = np.concatenate([cosT] * 4, axis=0)
    s128 = np.concatenate([-sinT, sinT, -sinT, sinT], axis=0)
    biasb = np.broadcast_to(bproj.astype(np.float32), (128, C)).copy()

    in_maps = []
    for core in range(NCORES):
        xs = x[core * BPC:(core + 1) * BPC]
        xT = np.ascontiguousarray(xs.transpose(0, 2, 1), dtype=np.float32)
        xTp = np.zeros((BPC, C, NP), dtype=np.float32)
        xTp[:, :, 0:N] = xT
        in_maps.append({
            "xT": xTp,
            "xTv": xT.astype(ml_dtypes.bfloat16),
            "wqkT": wqkT,
            "wvT": wvT,
            "wpT": wpT,
            "c128": c128,
            "s128": s128,
            "biasb": biasb,
        })
    return in_maps


def kernel(x, wqkv, wproj, bproj, freqs_cos, freqs_sin, trace=False):
    global _CACHED_NC, LAST_RESULTS
    if _CACHED_NC is None:
        _CACHED_NC = _build_nc()
    in_maps = _prep_inputs(x, wqkv, wproj, bproj, freqs_cos, freqs_sin)
    res = run_bass_kernel_spmd(_CACHED_NC, in_maps,
                               core_ids=list(range(NCORES)), trace=trace)
    LAST_RESULTS = res
    out = np.concatenate([r["out"] for r in res.results], axis=0)
    return out.astype(np.float32)


# revision 36
# speedup vs baseline: 2.2858x; 1.2244x over previous
"""PerformerAttention (standard softmax attention + interleaved RoPE) on 8 trn2 cores.

Sharding: data-parallel over batch (16 batches -> 2 per core), weights replicated.

Math per core (b = 2 local batches):
  qkv^T = wqkv @ x^T          (PE, fp32 for q/k path; V computed separately in bf16)
  RoPE on q,k (tokens 1..576) -- the even/odd interleave is pre-permuted into
    the wqkv rows host-side so on-chip rope uses contiguous 32-row blocks.
  S = (q/8)^T k  -> softmax over free dim  (fp32 scores: argmax-critical)
  P transposed via PE; out^T = V^T-style accumulation  (bf16)
  final = out^T.T @ wproj^T + bias  (bf16 matmul, fp32 accum)

Host-side prep (numpy): shard x, transpose x/w, permute wqkv q/k rows,
build cos/sin tiles, broadcast bias, cast bf16 operands.
"""

import numpy as np
import ml_dtypes

import concourse.bass as bass
import concourse.mybir as mybir
import concourse.tile as tile
from concourse import bacc
from concourse.bass_utils import run_bass_kernel_spmd
from concourse.masks import make_identity

F32 = mybir.dt.float32
F32R = mybir.dt.float32r
BF16 = mybir.dt.bfloat16
AX = mybir.AxisListType.X
COPY = mybir.ActivationFunctionType.Copy
EXP = mybir.ActivationFunctionType.Exp

B, N, C, H, D = 16, 577, 768, 12, 64
NCORES = 8
BPC = B // NCORES  # batches per core
NM1 = N - 1  # 576, rope'd tokens
NP = 578  # padded N (fp32r needs even free sizes)
NT = [(0, 128), (128, 128), (256, 128), (384, 128), (512, 65)]  # n tiles
MCH = [(0, 512), (512, 65)]  # free-dim chunks (psum bank = 512 fp32)
SCH = [(0, 320), (320, 258)]  # fp32r chunks (>=256 for 1 cycle/row, even)
FCH = [(0, 384), (384, 384)]  # proj/V output chunks

_CACHED_NC = None
LAST_RESULTS = None  # test harness reads exec_time_ns off this


def _build_nc():
    nc = bacc.Bacc("TRN2", target_bir_lowering=False)

    xT_d = nc.dram_tensor("xT", [BPC, C, NP], F32R, kind="ExternalInput")
    xTv_d = nc.dram_tensor("xTv", [BPC, C, N], BF16, kind="ExternalInput")
    wqk_d = nc.dram_tensor("wqkT", [C, 2 * C], F32R, kind="ExternalInput")
    wv_d = nc.dram_tensor("wvT", [C, C], BF16, kind="ExternalInput")
    wp_d = nc.dram_tensor("wpT", [C, C], BF16, kind="ExternalInput")
    c_d = nc.dram_tensor("c128", [128, NM1], F32, kind="ExternalInput")
    s_d = nc.dram_tensor("s128", [128, NM1], F32, kind="ExternalInput")
    bias_d = nc.dram_tensor("biasb", [128, C], F32, kind="ExternalInput")
    out_d = nc.dram_tensor("out", [BPC, N, C], F32, kind="ExternalOutput")

    with tile.TileContext(nc) as tc:
        with (
            tc.tile_pool(name="const", bufs=1) as constp,
            tc.tile_pool(name="persist", bufs=1) as pers,
            tc.tile_pool(name="work", bufs=2) as work,
            tc.tile_pool(name="ppool", bufs=6) as ppool,
            tc.tile_pool(name="ptpool", bufs=5) as ptpool,
            tc.tile_pool(name="obp", bufs=3) as obp,
            tc.tile_pool(name="stp", bufs=10) as stp,
            tc.tile_pool(name="psum", bufs=4, space="PSUM") as psp,
        ):
            # ---- constants / weights (once per core) ----
            ident = constp.tile([128, 128], BF16, name="ident", tag="ident")
            make_identity(nc, ident)

            wqk = []
            wv = []
            wp = []
            for ct in range(6):
                t = constp.tile([128, 2 * C], F32R, name=f"wqk{ct}", tag=f"wqk{ct}")
                nc.sync.dma_start(t, wqk_d[ct * 128:(ct + 1) * 128, :])
                wqk.append(t)
                t = constp.tile([128, C], BF16, name=f"wv{ct}", tag=f"wv{ct}")
                nc.sync.dma_start(t, wv_d[ct * 128:(ct + 1) * 128, :])
                wv.append(t)
                t = constp.tile([128, C], BF16, name=f"wp{ct}", tag=f"wp{ct}")
                nc.sync.dma_start(t, wp_d[ct * 128:(ct + 1) * 128, :])
                wp.append(t)
            c128 = constp.tile([128, NM1], F32, name="c128", tag="c128")
            nc.sync.dma_start(c128, c_d[:, :])
            s128 = constp.tile([128, NM1], F32, name="s128", tag="s128")
            nc.sync.dma_start(s128, s_d[:, :])
            biasb = constp.tile([128, C], F32, name="biasb", tag="biasb")
            nc.sync.dma_start(biasb, bias_d[:, :])

            # persistent per-batch tiles
            xT = [pers.tile([128, NP], F32R, name=f"xT{ct}", tag=f"xT{ct}")
                  for ct in range(6)]
            xTv = [pers.tile([128, N], BF16, name=f"xTv{ct}", tag=f"xTv{ct}")
                   for ct in range(6)]
            qkT = [pers.tile([128, NP], F32R, name=f"qkT{ft}", tag=f"qkT{ft}")
                   for ft in range(12)]
            V = [pers.tile([128, C], BF16, name=f"V{nt}", tag=f"V{nt}")
                 for nt in range(5)]
            attnT = [pers.tile([128, N], BF16, name=f"attnT{ct}", tag=f"attnT{ct}")
                     for ct in range(6)]

            for b in range(BPC):
                # ---- load x^T (fp32 + bf16 copies) ----
                for ct in range(6):
                    nc.sync.dma_start(xT[ct], xT_d[b, ct * 128:(ct + 1) * 128, :])
                    nc.sync.dma_start(xTv[ct], xTv_d[b, ct * 128:(ct + 1) * 128, :])

                # ---- qk^T = wqk^T.T @ x^T   [f,n] fp32r ----
                for ft in range(12):
                    for (cs, cw) in SCH:
                        ps = psp.tile([128, 512], F32, name="ps_qk", tag="mm")
                        for ct in range(6):
                            nc.tensor.matmul(
                                ps[:, 0:cw],
                                lhsT=wqk[ct][:, ft * 128:(ft + 1) * 128],
                                rhs=xT[ct][:, cs:cs + cw],
                                start=(ct == 0), stop=(ct == 5),
                            )
                        nc.scalar.activation(
                            qkT[ft][:, cs:cs + cw], ps[:, 0:cw], COPY,
                            bias=0.0, scale=1.0,
                        )

                # ---- RoPE on q,k tiles (cols 1..576), blocks [te;to;te;to] ----
                for ft in range(12):
                    t = qkT[ft]
                    tsw = work.tile([128, NM1], F32, name="tsw", tag="tsw")
                    rot1 = work.tile([128, NM1], F32, name="rot1", tag="rot1")
                    # tsw = [to0; te0; to1; te1]
                    nc.vector.tensor_copy(tsw[0:32, :], t[32:64, 1:N])
                    nc.vector.tensor_copy(tsw[32:64, :], t[0:32, 1:N])
                    nc.vector.tensor_copy(tsw[64:96, :], t[96:128, 1:N])
                    nc.vector.tensor_copy(tsw[96:128, :], t[64:96, 1:N])
                    nc.vector.tensor_mul(rot1, t[:, 1:N], c128)
                    nc.vector.tensor_mul(tsw, tsw, s128)  # s128 = [-s;s;-s;s]
                    nc.vector.tensor_add(t[:, 1:N], rot1, tsw)

                # ---- V = x @ wv^T   [n,f] bf16 ----
                for (ns, nsz) in NT:
                    nt = ns // 128
                    for (fs, fw) in FCH:
                        ps = psp.tile([128, 512], F32, name="ps_v", tag="mm")
                        for ct in range(6):
                            nc.tensor.matmul(
                                ps[0:nsz, 0:fw],
                                lhsT=xTv[ct][:, ns:ns + nsz],
                                rhs=wv[ct][:, fs:fs + fw],
                                start=(ct == 0), stop=(ct == 5),
                            )
                        nc.vector.tensor_copy(V[nt][0:nsz, fs:fs + fw],
                                              ps[0:nsz, 0:fw])

                # ---- attention per head ----
                for h in range(12):
                    qt = qkT[h // 2][(h % 2) * 64:(h % 2) * 64 + 64, :]
                    kt = qkT[6 + h // 2][(h % 2) * 64:(h % 2) * 64 + 64, :]

                    P = []
                    for (ns, nsz) in NT:
                        st = stp.tile([128, 8], F32, name="st", tag="st")
                        psA = psp.tile([128, 512], F32, name="ps_sA", tag="mm")
                        psB = psp.tile([128, 512], F32, name="ps_sB", tag="mm")
                        nc.tensor.matmul(psA[0:nsz, 0:320],
                                         lhsT=qt[:, ns:ns + nsz],
                                         rhs=kt[:, 0:320],
                                         start=True, stop=True)
                        nc.tensor.matmul(psB[0:nsz, 0:258],
                                         lhsT=qt[:, ns:ns + nsz],
                                         rhs=kt[:, 320:578],
                                         start=True, stop=True)
                        nc.vector.reduce_max(st[0:nsz, 0:1], psA[0:nsz, 0:320],
                                             axis=AX)
                        nc.vector.reduce_max(st[0:nsz, 1:2], psB[0:nsz, 0:257],
                                             axis=AX)
                        nc.vector.tensor_max(st[0:nsz, 2:3], st[0:nsz, 0:1],
                                             st[0:nsz, 1:2])
                        nc.vector.tensor_scalar_mul(st[0:nsz, 3:4],
                                                    st[0:nsz, 2:3], -1.0)
                        Pt = ppool.tile([128, N], BF16, name="P", tag="P")
                        nc.scalar.activation(Pt[0:nsz, 0:320], psA[0:nsz, 0:320],
                                             EXP, bias=st[0:nsz, 3:4],
                                             accum_out=st[0:nsz, 4:5])
                        nc.scalar.activation(Pt[0:nsz, 320:577], psB[0:nsz, 0:257],
                                             EXP, bias=st[0:nsz, 3:4],
                                             accum_out=st[0:nsz, 5:6])
                        nc.vector.tensor_add(st[0:nsz, 6:7], st[0:nsz, 4:5],
                                             st[0:nsz, 5:6])
                        nc.vector.reciprocal(st[0:nsz, 7:8], st[0:nsz, 6:7])
                        nc.vector.tensor_scalar_mul(Pt[0:nsz, :], Pt[0:nsz, :],
                                                    st[0:nsz, 7:8])
                        P.append(Pt)

                    # transpose P -> PT[mt] [m, n] bf16
                    PT = []
                    for (ms, msz) in NT:
                        psT = psp.tile([128, 512], BF16, name="psT", tag="tr",
                                       bufs=2)
                        psT4 = psp.tile([128, 128], BF16, name="psT4", tag="tr2",
                                        bufs=2)
                        for (ns, nsz) in NT:
                            nt = ns // 128
                            dst = (psT[0:msz, ns:ns + nsz] if nt < 4
                                   else psT4[0:msz, 0:nsz])
                            nc.tensor.transpose(dst, P[nt][0:nsz, ms:ms + msz],
                                                ident[0:nsz, 0:nsz])
                        PTt = ptpool.tile([128, N], BF16, name="PT", tag="PT")
                        nc.scalar.activation(PTt[0:msz, 0:512], psT[0:msz, :],
                                             COPY, bias=0.0, scale=1.0)
                        nc.scalar.activation(PTt[0:msz, 512:577],
                                             psT4[0:msz, 0:65],
                                             COPY, bias=0.0, scale=1.0)
                        PT.append(PTt)

                    # out^T[h] = sum_m V[m,:]^T-style: lhsT=V tile, rhs=PT
                    for (cs, cw) in MCH:
                        pso = psp.tile([128, 512], F32, name="ps_o", tag="mm")
                        for (ms, msz) in NT:
                            mt = ms // 128
                            nc.tensor.matmul(
                                pso[0:64, 0:cw],
                                lhsT=V[mt][0:msz, h * 64:h * 64 + 64],
                                rhs=PT[mt][0:msz, cs:cs + cw],
                                start=(mt == 0), stop=(mt == 4),
                            )
                        nc.scalar.activation(
                            attnT[h // 2][(h % 2) * 64:(h % 2) * 64 + 64,
                                          cs:cs + cw],
                            pso[0:64, 0:cw], COPY, bias=0.0, scale=1.0)

                # ---- proj + bias ----
                for (ns, nsz) in NT:
                    ob = obp.tile([128, C], F32, name="ob", tag="ob")
                    for (fs, fw) in FCH:
                        ps = psp.tile([128, 512], F32, name="ps_p", tag="mm")
                        for ct in range(6):
                            nc.tensor.matmul(
                                ps[0:nsz, 0:fw],
                                lhsT=attnT[ct][:, ns:ns + nsz],
                                rhs=wp[ct][:, fs:fs + fw],
                                start=(ct == 0), stop=(ct == 5),
                            )
                        nc.vector.tensor_add(ob[0:nsz, fs:fs + fw],
                                             ps[0:nsz, 0:fw],
                                             biasb[0:nsz, fs:fs + fw])
                    nc.sync.dma_start(out_d[b, ns:ns + nsz, :], ob[0:nsz, :])

    nc.compile()
    return nc


def _rope_perm():
    idx = []
    for h in range(H):
        base = h * D
        idx.extend(base + 2 * i for i in range(D // 2))      # evens
        idx.extend(base + 2 * i + 1 for i in range(D // 2))  # odds
    return np.array(idx)


def _prep_inputs(x, wqkv, wproj, bproj, freqs_cos, freqs_sin):
    perm = _rope_perm()
    wq = wqkv[0:C][perm] * 0.125
    wk = wqkv[C:2 * C][perm]
    wqkT = np.ascontiguousarray(np.concatenate([wq, wk], axis=0).T,
                                dtype=np.float32)
    wvT = np.ascontiguousarray(wqkv[2 * C:].T).astype(ml_dtypes.bfloat16)
    wpT = np.ascontiguousarray(wproj.T).astype(ml_dtypes.bfloat16)
    cosT = np.ascontiguousarray(freqs_cos.T, dtype=np.float32)  # [32, 576]
    sinT = np.ascontiguousarray(freqs_sin.T, dtype=np.float32)
    c128 = np.concatenate([cosT] * 4, axis=0)
    s128 = np.concatenate([-sinT, sinT, -sinT, sinT], axis=0)
    biasb = np.broadcast_to(bproj.astype(np.float32), (128, C)).copy()

    in_maps = []
    for core in range(NCORES):
        xs = x[core * BPC:(core + 1) * BPC]
        xT = np.ascontiguousarray(xs.transpose(0, 2, 1), dtype=np.float32)
        xTp = np.zeros((BPC, C, NP), dtype=np.float32)
        xTp[:, :, 0:N] = xT
        in_maps.append({
            "xT": xTp,
            "xTv": xT.astype(ml_dtypes.bfloat16),
            "wqkT": wqkT,
            "wvT": wvT,
            "wpT": wpT,
            "c128": c128,
            "s128": s128,
            "biasb": biasb,
        })
    return in_maps


def kernel(x, wqkv, wproj, bproj, freqs_cos, freqs_sin, trace=False):
    global _CACHED_NC, LAST_RESULTS
    if _CACHED_NC is None:
        _CACHED_NC = _build_nc()
    in_maps = _prep_inputs(x, wqkv, wproj, bproj, freqs_cos, freqs_sin)
    res = run_bass_kernel_spmd(_CACHED_NC, in_maps,
                               core_ids=list(range(NCORES)), trace=trace)
    LAST_RESULTS = res
    out = np.concatenate([r["out"] for r in res.results], axis=0)
    return out.astype(np.float32)
